# revision 19
# baseline (speedup 1.0000x reference)
"""Trainium2 Bass kernel for nn_Correlation (B=32, C=256, N=1024).

Reference pipeline per batch element:
  bypass = ReLU(BN(conv_bp(x)))                         [B, C, N]
  res    = cosine-similarity gram matrix of x           [B, N, N]
  h1     = ReLU(BN(conv1(res)))                         [B, C, N]
  h2     = ReLU(BN(conv2(cat(h1, bypass))))             [B, C, N]

Key restructuring: res = x_hat^T x_hat (x_hat = column-normalized x;
the diagonal force-to-1 differs from ||x_hat_i||^2 by ~1e-7 and is
dropped).  conv1(res) is then factored by associativity:

  conv1(res)[:, j] = sum_k W1_k res[:, j+k-1]
                   = sum_k (W1_k x_hat^T) x_hat[:, j+k-1]
                   = conv1d_k3(x_hat, B)   with B_k = W1_k @ x_hat^T

B_k is [C, C] per tap, so this replaces the N*N*C gram (268 MMAC) +
C*N*3*N conv1 (805 MMAC) with two C-sized stages (201 MMAC each),
cutting total PE work ~40%.  B_k is computed with x_hat^T as the
stationary operand (x^T is pre-transposed on the host; normalization
of both copies happens on-chip), producing B_k directly in the [c,
c_out] layout stage 2 needs as its stationary operand.

All matmul operands are bf16 (fp32 PSUM accumulation); measured model
error vs the fp32 reference is ~4e-3, well under the 2e-2 gate.
Outputs are DMA'd out as bf16 and upcast on the host.

Sharding: data-parallel over batch, 4 batches per core on 8 cores.
Training-mode BatchNorm statistics span the whole batch, so per-channel
partial (mean, E[x^2]) are AllReduced across the 8 cores.  Three small
AllReduces; the first two are latency-hidden (AR1 under the B/stage-2
phase, AR2 under conv2's bypass-half matmuls), AR3 is the exposed tail.

Conv1d(k=3, pad=1) is expressed as 3 shifted matmuls accumulating in
PSUM, using padded SBUF tiles with zero columns at both ends.

Conv biases are dropped: BN immediately subtracts the per-channel mean,
so a per-channel bias added before BN has exactly zero effect.
"""

import numpy as np
import ml_dtypes

import concourse.bass as bass
import concourse.mybir as mybir
import concourse.tile as tile
from concourse import bacc
from concourse.bass_utils import run_bass_kernel_spmd

P = 128
B = 32          # full batch
C = 256         # channels
N = 1024        # length (and gram size)
NCORES = 8
B_LOC = B // NCORES
NPAD = N + 2    # padded free dim: col 0 and col N+1 are zeros
CB = C // P     # channel blocks (2)
IB = N // P     # row blocks of the transposed x (8)
NCH = 2         # 512-wide chunks per row
CHW = N // NCH  # 512
F32 = mybir.dt.float32
BF = mybir.dt.bfloat16
AF = mybir.ActivationFunctionType
ALU = mybir.AluOpType
BN_EPS = 1e-5


def _build_kernel(sim_mode=False, debug=False):
    nc = bacc.Bacc(
        "TRN2",
        target_bir_lowering=False,
        debug=False,
        num_devices=1 if sim_mode else NCORES,
    )
    nc._sim_mode = sim_mode
    x_d = nc.dram_tensor("x", [B_LOC, CB, P, N], BF, kind="ExternalInput")
    xt_d = nc.dram_tensor("xt", [B_LOC, P, IB, C], BF, kind="ExternalInput")
    wbpt_d = nc.dram_tensor("wbpt", [P, 3, CB, C], BF, kind="ExternalInput")
    w1t_d = nc.dram_tensor("w1t", [P, IB, 3, C], BF, kind="ExternalInput")
    w2t_d = nc.dram_tensor("w2t", [P, 3, 2 * CB, C], BF, kind="ExternalInput")
    bnp_d = nc.dram_tensor("bnp", [P, 6, CB], F32, kind="ExternalInput")
    out_d = nc.dram_tensor("out", [B_LOC, CB, P, N], BF, kind="ExternalOutput")
    dbg = None
    if debug:
        dbg = {
            "d_byp": nc.dram_tensor("d_byp", [CB, P, NPAD], BF, kind="ExternalOutput"),
            "d_xhat": nc.dram_tensor("d_xhat", [CB, P, NPAD], BF, kind="ExternalOutput"),
            "d_xt": nc.dram_tensor("d_xt", [P, IB, C], BF, kind="ExternalOutput"),
            "d_bsb": nc.dram_tensor("d_bsb", [CB, P, 3, C], BF, kind="ExternalOutput"),
            "d_h1": nc.dram_tensor("d_h1", [CB, P, NPAD], BF, kind="ExternalOutput"),
            "d_h2": nc.dram_tensor("d_h2", [CB, P, N], F32, kind="ExternalOutput"),
            "d_scal": nc.dram_tensor("d_scal", [P, 3, CB, 2], F32, kind="ExternalOutput"),
        }

    with tile.TileContext(nc) as tc:
        _kernel_body(
            tc,
            x_d.ap(),
            xt_d.ap(),
            wbpt_d.ap(),
            w1t_d.ap(),
            w2t_d.ap(),
            bnp_d.ap(),
            out_d.ap(),
            dbg={k: v.ap() for k, v in dbg.items()} if dbg else None,
        )
    nc.compile()
    return nc


def _kernel_body(tc, x, xt, wbpt, w1t, w2t, bnp, out, dbg=None):
    nc = tc.nc

    with (
        tc.tile_pool(name="data", bufs=1) as data,
        tc.tile_pool(name="big", bufs=1) as big,
        tc.tile_pool(name="psA", bufs=4, space="PSUM") as psA,
        tc.tile_pool(name="psB", bufs=4, space="PSUM") as psB,
        tc.tile_pool(name="dramp", bufs=1, space="DRAM") as dramp,
    ):
        def zero_pads(t):
            nc.gpsimd.memset(t[:, 0:1], 0)
            nc.gpsimd.memset(t[:, NPAD - 1 : NPAD], 0)

        # ---- input DMAs, ordered so the first bypass matmuls can start
        # as early as possible -----------------------------------------
        xs = {}

        def load_xs_tile(b, ct):
            t = big.tile([P, NPAD], BF, tag="xs", bufs=8, name=f"xs_{b}_{ct}")
            zero_pads(t)
            nc.sync.dma_start(t[:, 1 : 1 + N], x[b, ct])
            xs[b, ct] = t

        wbpt_sb = data.tile([P, 3, CB, C], BF)
        nc.sync.dma_start(wbpt_sb[:, :, :, 0:P], wbpt[:, :, :, 0:P])
        load_xs_tile(0, 0)
        load_xs_tile(0, 1)
        nc.sync.dma_start(wbpt_sb[:, :, :, P:C], wbpt[:, :, :, P:C])
        bnp_sb = data.tile([P, 6, CB], F32)
        nc.sync.dma_start(bnp_sb[:], bnp[:])
        for b in range(1, B_LOC):
            load_xs_tile(b, 0)
            load_xs_tile(b, 1)
        xt_sb = {}
        for b in range(B_LOC):
            t = data.tile([P, IB, C], BF, name=f"xt_{b}")
            nc.gpsimd.dma_start(t[:], xt[b])
            xt_sb[b] = t
        w1t_sb = data.tile([P, IB, 3, C], BF)
        nc.gpsimd.dma_start(w1t_sb[:], w1t[:])
        w2t_sb = data.tile([P, 3, 2 * CB, C], BF)
        nc.gpsimd.dma_start(w2t_sb[:], w2t[:])

        ones_col = data.tile([P, 1], BF)    # lhsT for row-sum matmuls
        nc.gpsimd.memset(ones_col[:], 1.0)
        ones_row = data.tile([1, P], BF)    # lhsT for partition-broadcast mm
        nc.gpsimd.memset(ones_row[:], 1.0)
        eps_sb = data.tile([P, 1], F32)     # BN epsilon as a bias AP
        nc.gpsimd.memset(eps_sb[:], BN_EPS)

        # per-chunk BN statistics: [P, cb, b*2+ch, 6]
        stats_bp = data.tile([P, CB, 2 * B_LOC, 6], F32)
        stats_h1 = data.tile([P, CB, 2 * B_LOC, 6], F32)
        stats_h2 = data.tile([P, CB, 2 * B_LOC, 6], F32)
        # final per-channel affine (a, b) for the three BN+ReLU stages
        scal = data.tile([P, 3, CB, 2], F32)

        byp = {}
        h1 = {}
        h2 = {}
        for b in range(B_LOC):
            for cb in range(CB):
                t = big.tile([P, NPAD], BF, tag="byp", bufs=8, name=f"byp_{b}_{cb}")
                zero_pads(t)
                byp[b, cb] = t
                t = big.tile([P, NPAD], BF, tag="h1", bufs=8, name=f"h1_{b}_{cb}")
                zero_pads(t)
                h1[b, cb] = t
                h2[b, cb] = big.tile([P, N], F32, tag="h2", bufs=8, name=f"h2_{b}_{cb}")

        def copy_psum(dst, ps, i):
            # split PSUM->SBUF copies between ScalarE and VectorE
            if i % 2 == 0:
                nc.scalar.activation(dst, ps, AF.Copy)
            else:
                nc.vector.tensor_copy(dst, ps)

        # ---- BN statistics: local aggregate -> AllReduce -> (a, b) ------
        def bn_kickoff(mv, tag):
            payload = data.tile([P, CB, 2], F32, name=f"payload_{tag}")
            # payload[...,0] = mean ; payload[...,1] = mean^2 + var = E[x^2]
            nc.gpsimd.tensor_copy(payload[:, :, 0], mv[:, :, 0])
            nc.gpsimd.tensor_mul(payload[:, :, 1], mv[:, :, 0], mv[:, :, 0])
            nc.gpsimd.tensor_add(payload[:, :, 1], payload[:, :, 1], mv[:, :, 1])
            nelem = CB * 2
            ar_in = dramp.tile([P, nelem], F32, name=f"ar_in_{tag}")
            ar_out = dramp.tile([P, nelem], F32, name=f"ar_out_{tag}")
            nc.gpsimd.dma_start(ar_in[:], payload[:].rearrange("p a b -> p (a b)"))
            if getattr(nc, "_sim_mode", False):
                nc.gpsimd.dma_start(ar_out[:], ar_in[:])
            else:
                nc.gpsimd.collective_compute(
                    "AllReduce",
                    ALU.add,
                    replica_groups=[list(range(NCORES))],
                    ins=[ar_in[:].opt()],
                    outs=[ar_out[:].opt()],
                )
            return ar_out

        def bn_reduce(ci, st, tag):
            """Aggregate per-chunk stats, kick off the AllReduce of
            per-channel (mean, E[x^2]); returns the AR output dram tile."""
            mv = data.tile([P, CB, 2], F32, name=f"mv_{tag}")
            for cb in range(CB):
                nc.vector.bn_aggr(
                    mv[:, cb, :],
                    st[:, cb].rearrange("p a b -> p (a b)"),
                )
            return bn_kickoff(mv, tag)

        def bn_finalize(ar_out, ci, tag):
            """DMA the AllReduced stats back and compute scal[ci] = (a, b)."""
            gst = data.tile([P, CB, 2], F32, name=f"gst_{tag}")
            nc.scalar.dma_start(gst[:].rearrange("p a b -> p (a b)"), ar_out[:])
            # gst holds SUMS over the 8 cores:
            #   gms = 8*mean, ex2s = 8*E[x^2]
            #   var = (8*ex2s - gms^2) / 64 ; sqrt folds the /64 + eps in
            gms = gst[:, :, 0]
            ex2s = gst[:, :, 1]
            u = data.tile([P, CB], F32, name=f"u_{tag}")
            nc.vector.tensor_mul(u[:], gms, gms)
            nc.vector.scalar_tensor_tensor(
                u[:], ex2s, float(NCORES), u[:], ALU.mult, ALU.subtract
            )
            sd = data.tile([P, CB], F32, name=f"sd_{tag}")
            nc.scalar.activation(
                sd[:], u[:], AF.Sqrt, bias=eps_sb[:], scale=1.0 / (NCORES * NCORES)
            )
            rstd = data.tile([P, CB], F32, name=f"rstd_{tag}")
            nc.vector.reciprocal(rstd[:], sd[:])
            # bnp layout: [P, 2*ci + {0:gamma,1:beta}, cb]
            gamma = bnp_sb[:, 2 * ci, :]
            beta = bnp_sb[:, 2 * ci + 1, :]
            a_all = scal[:, ci, :, 0]
            b_all = scal[:, ci, :, 1]
            nc.vector.tensor_mul(a_all, gamma, rstd[:])
            nc.vector.scalar_tensor_tensor(
                rstd[:], gms, 1.0 / NCORES, a_all, ALU.mult, ALU.mult
            )
            nc.vector.tensor_sub(b_all, beta, rstd[:])

        # ---- norm prologue: row norms + in-place column scaling ---------
        # After this, xs[b] holds x / ||x_j||.  (The transposed copy is
        # normalized independently in the B phase below.)
        def _prologue_steps(b, state):
            """Generator of prologue steps for batch b, interleaved into
            the matmul emission so no engine queue floods."""
            for ct in range(CB):
                t = big.tile([P, N], BF, tag="xsq", bufs=2, name=f"xsq_{ct}")
                nc.vector.tensor_mul(
                    t[:], xs[b, ct][:, 1 : 1 + N], xs[b, ct][:, 1 : 1 + N]
                )
                state.setdefault("xsq", []).append(t)
                yield
            xsq = state["xsq"]
            s_row = big.tile([1, N], F32, tag="srow", bufs=2, name="s_row")
            for ch in range(NCH):
                psr = psB.tile([1, CHW], F32, tag="pb1", bufs=2, name="ps_row")
                for ct in range(CB):
                    nc.tensor.matmul(
                        psr[:],
                        ones_col[:],
                        xsq[ct][:, ch * CHW : ch * CHW + CHW],
                        start=(ct == 0),
                        stop=(ct == CB - 1),
                    )
                nc.scalar.sqrt(s_row[0:1, ch * CHW : ch * CHW + CHW], psr[:])
            inv_row = big.tile([1, N], BF, tag="irow", bufs=2, name="inv_row")
            with nc.allow_low_precision(reason="bf16 rounding of inv-norms ok"):
                nc.vector.reciprocal(inv_row[0:1, :], s_row[0:1, :])
            state["inv_row"] = inv_row
            yield
            inv_row = state["inv_row"]
            psbs = []
            for ch in range(NCH):
                psb = psB.tile([P, CHW], F32, tag="pb1", bufs=2, name="ps_invb")
                nc.tensor.matmul(
                    psb[:],
                    ones_row[:],
                    inv_row[0:1, ch * CHW : ch * CHW + CHW],
                    start=True,
                    stop=True,
                )
                psbs.append(psb)
            state["psbs"] = psbs
            yield
            for ch in range(NCH):
                for ct in range(CB):
                    nc.vector.tensor_mul(
                        xs[b, ct][:, 1 + ch * CHW : 1 + ch * CHW + CHW],
                        xs[b, ct][:, 1 + ch * CHW : 1 + ch * CHW + CHW],
                        state["psbs"][ch][:],
                    )
                    yield

        prologues = [_prologue_steps(b, {}) for b in range(B_LOC)]

        def xt_normalize(b):
            sqt = big.tile([P, IB, C], BF, tag="sqt", bufs=2, name="sqt")
            nc.vector.tensor_mul(sqt[:], xt_sb[b][:], xt_sb[b][:])
            nrm2 = big.tile([P, IB], F32, tag="nrm2", bufs=2, name="nrm2")
            nc.vector.tensor_reduce(nrm2[:], sqt[:], mybir.AxisListType.X, ALU.add)
            nrmt = big.tile([P, IB], F32, tag="nrmt", bufs=2, name="nrmt")
            nc.scalar.sqrt(nrmt[:], nrm2[:])
            invt = big.tile([P, IB], F32, tag="invt", bufs=2, name="invt")
            nc.vector.reciprocal(invt[:], nrmt[:])
            for ib in range(IB):
                nc.vector.tensor_scalar(
                    xt_sb[b][:, ib, :],
                    xt_sb[b][:, ib, :],
                    invt[:, ib : ib + 1],
                    None,
                    ALU.mult,
                )

        def prologue_step(b, n=1):
            for _ in range(n):
                next(prologues[b], None)

        # ---- phase A: bypass conv ---------------------------------------
        for b in range(B_LOC):
            for cb in range(CB):
                for ch in range(NCH):
                    ps = psA.tile([P, CHW], F32, tag="g", name="ps_bp")
                    for ct in range(CB):
                        for k in range(3):
                            nc.tensor.matmul(
                                ps[:],
                                wbpt_sb[:, k, ct, cb * P : (cb + 1) * P],
                                xs[b, ct][:, k + ch * CHW : k + ch * CHW + CHW],
                                start=(ct == 0 and k == 0),
                                stop=(ct == CB - 1 and k == 2),
                            )
                    nc.vector.bn_stats(stats_bp[:, cb, 2 * b + ch, :], ps[:])
                    nc.scalar.activation(
                        byp[b, cb][:, 1 + ch * CHW : 1 + ch * CHW + CHW],
                        ps[:],
                        AF.Copy,
                    )
                # after bp(b) is done reading xs[b-1]... start normalizing
                # the PREVIOUS batch (its bypass reads are complete)
                if b >= 1:
                    prologue_step(b - 1, 5)
            xt_normalize(b)
        # finish all prologue steps not yet emitted (hidden under B phase)
        prologue_step(B_LOC - 1, 100)
        ar_bp = bn_reduce(0, stats_bp, "bp")
        if dbg is not None:
            for cb in range(CB):
                nc.sync.dma_start(dbg["d_byp"][cb], byp[0, cb][:])

        # ---- phase B: B_k = W1_k @ xhat^T, then stage-2 conv ------------
        # The transposed copy is normalized on the fly: row norms via a
        # DVE square+reduce over the free (channel) axis, then a
        # per-partition rsqrt scale.
        byp_apply_queue = [
            (bb, cb, ch) for bb in range(B_LOC) for cb in range(CB) for ch in range(NCH)
        ]

        def emit_next_byp_apply():
            if not byp_apply_queue:
                return
            bb, cb, ch = byp_apply_queue.pop(0)
            sl = slice(1 + ch * CHW, 1 + ch * CHW + CHW)
            nc.scalar.activation(
                byp[bb, cb][:, sl],
                byp[bb, cb][:, sl],
                AF.Relu,
                bias=scal[:, 0, cb, 1:2],
                scale=scal[:, 0, cb, 0:1],
            )

        bsb = {}

        def b_phase(b):
            pb1 = {}
            pb2 = {}
            for cb in range(CB):
                pb1[cb] = psB.tile([P, 2 * C], F32, tag="pb1", bufs=2, name="ps_b1")
                pb2[cb] = psB.tile([P, C], F32, tag="pb2", bufs=2, name="ps_b2")
            for ib in range(IB):
                for cb in range(CB):
                    lhs = xt_sb[b][:, ib, cb * P : (cb + 1) * P]
                    nc.tensor.matmul(
                        pb1[cb][:],
                        lhs,
                        w1t_sb[:, ib, 0:2, :],
                        start=(ib == 0),
                        stop=(ib == IB - 1),
                    )
                    nc.tensor.matmul(
                        pb2[cb][:],
                        lhs,
                        w1t_sb[:, ib, 2, :],
                        start=(ib == 0),
                        stop=(ib == IB - 1),
                    )
            for cb in range(CB):
                t = big.tile([P, 3, C], BF, tag="bsb", bufs=4, name=f"bsb_{cb}")
                copy_psum(t[:, 0:2, :], pb1[cb][:], 0)
                copy_psum(t[:, 2, :], pb2[cb][:], 1)
                bsb[b, cb] = t

        def s2_phase(b):
            # stage 2: h1_pre = conv1d_k3(xhat, B)
            for cb_out in range(CB):
                for ch in range(NCH):
                    ps = psA.tile([P, CHW], F32, tag="g", name="ps_s2")
                    for cbk in range(CB):
                        for k in range(3):
                            nc.tensor.matmul(
                                ps[:],
                                bsb[b, cbk][:, k, cb_out * P : (cb_out + 1) * P],
                                xs[b, cbk][:, k + ch * CHW : k + ch * CHW + CHW],
                                start=(cbk == 0 and k == 0),
                                stop=(cbk == CB - 1 and k == 2),
                            )
                    nc.vector.bn_stats(stats_h1[:, cb_out, 2 * b + ch, :], ps[:])
                    copy_psum(
                        h1[b, cb_out][:, 1 + ch * CHW : 1 + ch * CHW + CHW],
                        ps[:],
                        cb_out * NCH + ch,
                    )
            if b >= 1:
                for _ in range(6 if b == B_LOC - 1 else 5):
                    emit_next_byp_apply()

        # pipelined emission: B(b+1) ahead of stage2(b) so the PE always
        # has ready work while bsb copies / prologue tails drain
        b_phase(0)
        bn_finalize(ar_bp, 0, "bp")
        b_phase(1)
        s2_phase(0)
        b_phase(2)
        s2_phase(1)
        b_phase(3)
        s2_phase(2)
        s2_phase(3)

        while byp_apply_queue:
            emit_next_byp_apply()

        # AllReduce of conv1 stats — hidden under conv2's bypass-half MMs
        ar_h1 = bn_reduce(1, stats_h1, "h1")

        # ---- conv2: bypass-half first (independent of the AR) ----------
        groups = [
            (b, cb, ch)
            for b in range(B_LOC)
            for cb in range(CB)
            for ch in range(NCH)
        ]

        def conv2_half(ps, b, cb, ch, src, base):
            for ct in range(CB):
                for k in range(3):
                    nc.tensor.matmul(
                        ps[:],
                        w2t_sb[:, k, base + ct, cb * P : (cb + 1) * P],
                        src[b, ct][:, k + ch * CHW : k + ch * CHW + CHW],
                        start=(ct == 0 and k == 0),
                        stop=(ct == CB - 1 and k == 2),
                    )

        for gi, (b, cb, ch) in enumerate(groups):
            ps = psA.tile([P, CHW], F32, tag="g", name="ps_c2a")
            conv2_half(ps, b, cb, ch, byp, 2)
            copy_psum(h2[b, cb][:, ch * CHW : ch * CHW + CHW], ps[:], gi)

        bn_finalize(ar_h1, 1, "h1")
        # h1 BN+ReLU applies (in place)
        for i, (b, cb) in enumerate(
            (b, cb) for b in range(B_LOC) for cb in range(CB)
        ):
            if i % 2 == 0:
                nc.scalar.activation(
                    h1[b, cb][:, 1 : 1 + N],
                    h1[b, cb][:, 1 : 1 + N],
                    AF.Relu,
                    bias=scal[:, 1, cb, 1:2],
                    scale=scal[:, 1, cb, 0:1],
                )
            else:
                hap = h1[b, cb][:, 1 : 1 + N]
                nc.vector.tensor_scalar(
                    hap, hap, scal[:, 1, cb, 0:1], scal[:, 1, cb, 1:2],
                    ALU.mult, ALU.add,
                )
                nc.vector.tensor_scalar_max(hap, hap, 0.0)

        mv_h2 = data.tile([P, CB, 2], F32, name="mv_h2")
        groups_cb = [
            (cb, b, ch)
            for cb in range(CB)
            for b in range(B_LOC)
            for ch in range(NCH)
        ]
        for gi, (cb, b, ch) in enumerate(groups_cb):
            ps = psA.tile([P, CHW], F32, tag="g", name="ps_c2b")
            conv2_half(ps, b, cb, ch, h1, 0)
            hchunk = h2[b, cb][:, ch * CHW : ch * CHW + CHW]
            nc.vector.tensor_add(hchunk, hchunk, ps[:])
            nc.vector.bn_stats(stats_h2[:, cb, 2 * b + ch, :], hchunk)
            if gi % (2 * B_LOC) == 2 * B_LOC - 1:
                nc.vector.bn_aggr(
                    mv_h2[:, cb, :],
                    stats_h2[:, cb].rearrange("p a b -> p (a b)"),
                )

        # ---- BN statistics AllReduce #3 (conv2) and the output tail ----
        ar_h2 = bn_kickoff(mv_h2, "h2")
        bn_finalize(ar_h2, 2, "h2")
        if dbg is not None:
            for cb in range(CB):
                nc.sync.dma_start(dbg["d_h2"][cb], h2[0, cb][:])
            nc.sync.dma_start(dbg["d_scal"][:], scal[:])

        for idx, (b, cb, ch) in enumerate(
            (b, cb, ch) for b in range(B_LOC) for cb in range(CB) for ch in range(NCH)
        ):
            sl = slice(ch * CHW, ch * CHW + CHW)
            hap = h2[b, cb][:, sl]
            # stage the bf16 result in the (dead) xs tile for this slot
            stg = xs[b, cb][:, 1 + ch * CHW : 1 + ch * CHW + CHW]
            if idx % 2 == 1:
                nc.vector.tensor_scalar(
                    stg, hap, scal[:, 2, cb, 0:1], scal[:, 2, cb, 1:2],
                    ALU.mult, ALU.add,
                )
                nc.vector.tensor_scalar_max(stg, stg, 0.0)
            else:
                nc.scalar.activation(
                    stg, hap, AF.Relu,
                    bias=scal[:, 2, cb, 1:2], scale=scal[:, 2, cb, 0:1],
                )
            eng = nc.sync if idx % 2 == 0 else nc.gpsimd
            eng.dma_start(out[b, cb, :, sl], stg)


_NC_CACHE = None


def _get_nc():
    global _NC_CACHE
    if _NC_CACHE is None:
        _NC_CACHE = _build_kernel()
    return _NC_CACHE


def _prep_inputs(x, w_bp, w1, w2, g_bp, be_bp, g1, be1, g2, be2):
    bf16 = ml_dtypes.bfloat16
    xs = np.asarray(x, np.float32)[..., 0].astype(bf16)  # [B, C, N]
    xt = np.ascontiguousarray(xs.transpose(0, 2, 1))     # [B, N, C]
    xs = np.ascontiguousarray(xs)
    wbpt = np.ascontiguousarray(
        np.asarray(w_bp, np.float32).reshape(C, CB, P, 3).transpose(2, 3, 1, 0)
    ).astype(bf16)
    w1t = np.ascontiguousarray(
        np.asarray(w1, np.float32).reshape(C, IB, P, 3).transpose(2, 1, 3, 0)
    ).astype(bf16)
    w2t = np.ascontiguousarray(
        np.asarray(w2, np.float32).reshape(C, 2 * CB, P, 3).transpose(2, 3, 1, 0)
    ).astype(bf16)
    bnp = np.ascontiguousarray(
        np.stack([g_bp, be_bp, g1, be1, g2, be2])
        .astype(np.float32)
        .reshape(6, CB, P)
        .transpose(2, 0, 1)
    )
    in_maps = []
    for core in range(NCORES):
        shard = xs[core * B_LOC : (core + 1) * B_LOC].reshape(B_LOC, CB, P, N)
        shard_t = (
            xt[core * B_LOC : (core + 1) * B_LOC]
            .reshape(B_LOC, IB, P, C)
            .transpose(0, 2, 1, 3)
        )
        in_maps.append(
            {
                "x": np.ascontiguousarray(shard),
                "xt": np.ascontiguousarray(shard_t),
                "wbpt": wbpt,
                "w1t": w1t,
                "w2t": w2t,
                "bnp": bnp,
            }
        )
    return in_maps


def kernel(
    x,
    w_bp,
    b_bp,
    g_bp,
    be_bp,
    w1,
    b1,
    g1,
    be1,
    w2,
    b2,
    g2,
    be2,
    _want_results=False,
    **_unused,
):
    nc = _get_nc()
    in_maps = _prep_inputs(x, w_bp, w1, w2, g_bp, be_bp, g1, be1, g2, be2)
    res = run_bass_kernel_spmd(nc, in_maps, core_ids=list(range(NCORES)))
    outs = [
        r["out"].astype(np.float32).reshape(B_LOC, C, N) for r in res.results
    ]
    full = np.concatenate(outs, axis=0)[..., None]  # [B, C, N, 1]
    if _want_results:
        return full, res
    return full
